# revision 32
# baseline (speedup 1.0000x reference)
"""Trainium2 Bass kernel for a CamembertLayer (BERT encoder layer, no attn
output projection):  QKV -> attention -> +residual -> LN1 -> FFN(gelu) ->
+residual -> LN2.

Sharding: data-parallel over 8 cores.  Core c handles batch b=c//2, sequence
half h=c%2 (1024 query tokens).  K/V are computed redundantly over the full
2048-token sequence of the batch, so no collectives are needed.  The host
rotates each core's sequence so its query half is always rows 0..1023.

v3 design notes:
 - activations transposed ([H, tokens]); x^T arrives pre-transposed from the
   host in bf16 (xbT) and fp8 (x8T): no on-device x transposes at all.
 - Q/K projections in bf16 (fp8 scores cost ~1.5e-2 rel err: score errors
   scale with |s| and hit the softmax winners).  V is error-tolerant (damped
   by the residual) and is computed DIRECTLY in natural [kv, hd] layout for
   ALL heads upfront: fp8 DoubleRow with the x chunk as the stationary
   operand and Wv as the moving operand, accumulating [128kv x 8pairs*128]
   in two PSUM banks -- this kills the per-pair V^T projection, the PE
   V transposes, and their DVE copies, and frees 2 PSUM banks.
 - scores matmuls alternate heads tile-by-tile: the 64-row stationary
   (head-dim contraction) disables FWL, but alternating row-groups lets
   each LDWEIGHTS overlap the other head's matmul.  ps pool has 3 bufs so
   the ACT exp stream never starves.
 - exp with -3.6 bias (cancels in softmax; keeps fp8 < 240) -> fp8e4 probs;
   ctx matmul in fp8 DoubleRow against Vn (ones-column at col 64 folds the
   softmax denominator).
 - LN1/LN2 rstd = exp(-0.5*ln(var)) on ACT (exp/ln share one table set).
 - FFN: dense_1 streams W1 and gelu-writes the whole half's inter^T into
   one [128, 32, 512] buffer; dense_2 accumulates all 32 ff tiles into one
   PSUM bank per output chunk (one DVE add each).  W2 host-pre-rearranged.
 - LN2 runs in the transposed layout (ones-matmul stats like LN1) and the
   kernel emits a TRANSPOSED output [H, S_q]; the host transposes back.
   This removes the final PE transpose pass entirely.
"""
import sys

for _p in ("/opt/trn_rl_repo",):
    if _p not in sys.path:
        sys.path.insert(0, _p)

import numpy as np
from contextlib import ExitStack

import concourse.bass as bass
import concourse.bacc as bacc
import concourse.mybir as mybir
import concourse.tile as tile
from concourse.masks import make_identity

fp32 = mybir.dt.float32
fp32r = mybir.dt.float32r
bf16 = mybir.dt.bfloat16
fp8e4 = mybir.dt.float8e4
AF = mybir.ActivationFunctionType
ALU = mybir.AluOpType
PM = mybir.MatmulPerfMode

FULL_CFG = dict(H=1024, NH=16, FF=4096, S_kv=2048, S_q=1024, QB=512, FFC=512,
                act="gelu", ctx8=True)
EPS = 1e-12
HD = 64
EXPB = -3.6          # exp(s/8 + EXPB): cancels in softmax; keeps fp8 < 240
WSC = 64.0           # host scale on Wv fp8
XSC = 8.0            # host scale on x8T fp8


def build_nc(cfg):
    H, NH, FF = cfg["H"], cfg["NH"], cfg["FF"]
    S_kv, S_q, QB, FFC = cfg["S_kv"], cfg["S_q"], cfg["QB"], cfg["FFC"]
    Hc = H // 128          # hidden chunks of 128
    NP = NH // 2           # head pairs
    Tkv = S_kv // 128      # kv token tiles
    Tq = S_q // 128        # q token tiles
    NB = min(512, S_q)     # projection/stat block along q
    QNB = S_q // NB
    Fm = FFC // 128        # ff tiles per chunk
    NFC = FF // FFC        # ff chunks
    NFm = FF // 128        # all ff tiles
    act_fn = AF.Gelu if cfg.get("act", "gelu") == "gelu" else AF.Sigmoid
    ctx8 = cfg.get("ctx8", True)
    edt = fp8e4 if ctx8 else bf16       # expS / Vn dtype
    fdt = bf16

    nc = bacc.Bacc(num_swdge_queues=4)
    xbT = nc.declare_dram_parameter("xbT", [H, S_q], bf16, isOutput=False)
    x8T = nc.declare_dram_parameter("x8T", [H, S_kv], fp8e4, isOutput=False)
    WqR = nc.declare_dram_parameter("WqR", [128, NP * Hc * 128], bf16,
                                    isOutput=False)
    WkR = nc.declare_dram_parameter("WkR", [128, NP * Hc * 128], fp8e4,
                                    isOutput=False)
    Wv8 = nc.declare_dram_parameter("Wv8", [H, H], fp8e4, isOutput=False)
    bq = nc.declare_dram_parameter("bq", [H], fp32, isOutput=False)
    bk = nc.declare_dram_parameter("bk", [H], fp32, isOutput=False)
    bv = nc.declare_dram_parameter("bv", [H], fp32, isOutput=False)
    ln1_g = nc.declare_dram_parameter("ln1_g", [H], fp32, isOutput=False)
    ln1_b = nc.declare_dram_parameter("ln1_b", [H], fp32, isOutput=False)
    W1 = nc.declare_dram_parameter("W1", [H, FF], bf16, isOutput=False)
    b1 = nc.declare_dram_parameter("b1", [FF], fp32, isOutput=False)
    W2R = nc.declare_dram_parameter("W2R", [128, Hc * NFm * 128], fdt,
                                    isOutput=False)
    b2 = nc.declare_dram_parameter("b2", [H], fp32, isOutput=False)
    ln2_g = nc.declare_dram_parameter("ln2_g", [H], fp32, isOutput=False)
    ln2_b = nc.declare_dram_parameter("ln2_b", [H], fp32, isOutput=False)
    # transposed output [H, S_q]; host transposes back
    outT = nc.declare_dram_parameter("outT", [H, S_q], fp32, isOutput=True)

    dmac = nc.gpsimd.dma_start   # SWDGE: casts on the fly

    with tile.TileContext(nc) as tc, ExitStack() as ctx:
        pers = ctx.enter_context(tc.tile_pool(name="pers", bufs=1))

        ones_f = pers.tile([128, 128], fp32)
        nc.vector.memset(ones_f, 1.0)
        ones_col = pers.tile([128, 1], fp32r)
        nc.vector.tensor_copy(ones_col, ones_f[:, 0:1])
        ones_col_bf = pers.tile([128, 1], bf16)
        nc.vector.tensor_copy(ones_col_bf, ones_f[:, 0:1])
        expb_sb = pers.tile([128, 1], fp32, name="expb_sb")
        nc.vector.memset(expb_sb, EXPB)

        bq_sb = pers.tile([128, NP], fp32)
        bk_sb = pers.tile([128, NP], fp32)
        b1_sb = pers.tile([128, FF // 128], fp32)
        b2_sb = pers.tile([128, Hc], fp32)
        l1g_sb = pers.tile([128, Hc], fp32)
        l1b_sb = pers.tile([128, Hc], fp32)
        l2g_sb = pers.tile([128, Hc], fp32)
        l2b_sb = pers.tile([128, Hc], fp32)
        # persistent activations
        ctxT = pers.tile([128, Hc, S_q], fp32r)   # ctx^T, later s2^T
        sqall = pers.tile([128, Hc, QNB, NB], bf16)
        # LN1 mean/rstd broadcasts, computed at the attention tail
        pmu1 = pers.tile([128, QNB, NB], fp32, name="pmu1")
        prs1 = pers.tile([128, QNB, NB], fp32, name="prs1")

        # ---------------- Phase A+B: x^T, V, attention -------------------
        with tc.tile_pool(name="attn", bufs=1) as attn:
            xT = attn.tile([128, Hc, S_q], bf16)
            xT8 = attn.tile([128, Hc, S_kv], fp8e4, name="xT8")
            VW = 80
            Vn = attn.tile([128, Tkv, NH, VW], edt, name="Vn")
            wv_sb = attn.tile([128, Hc, H], fp8e4, name="wv_sb")
            bv_bc = attn.tile([128, H], fp32, name="bv_bc")
            # DMA order: V-phase deps first (x8T + Wv), residual/Q x last,
            # per-chunk dmacs spread across the SWDGE queues
            for c in range(Hc):
                dmac(out=xT8[:, c, :],
                     in_=x8T.ap()[c * 128:(c + 1) * 128, :])
            for c in range(0, Hc, 2):
                dmac(out=wv_sb[:, c:c + 2, :],
                     in_=Wv8.ap()[c * 128:(c + 2) * 128, :]
                     .rearrange("(c k) m -> k c m", k=128))
            dmac(out=bv_bc, in_=bass.AP(tensor=bv, offset=0,
                                        ap=[[0, 128], [1, H]]))
            for c in range(Hc):
                dmac(out=xT[:, c, :],
                     in_=xbT.ap()[c * 128:(c + 1) * 128, :])
            # V natural for ALL heads, computed directly as [kv, head, dim]:
            # Vn[:, t, head, 0:64] = 8*(V+bv) dims, col 64 = 8.0 (denom
            # fold: ratio of the ctx matmul rows is invariant to the 8x),
            # cols 65:80 zero pad (stride 80 for the DoubleRow step rule)
            nc.vector.memset(Vn[:, :, :, 64:VW], 0.0)
            nc.vector.memset(Vn[:, :, :, 64:65], 8.0)
            nc.vector.tensor_scalar_mul(bv_bc, bv_bc, XSC)   # 8*bv

            with tc.tile_pool(name="psV", bufs=2, space="PSUM") as psV:
                for t in range(Tkv):
                    ts = slice(t * 128, (t + 1) * 128)
                    pv = psV.tile([128, 1024], fp32, tag="pv", bufs=2)
                    for g in range(Hc // 2):
                        # stationary = x chunk (fp8), moving = Wv columns
                        for hh in range(2):
                            nc.tensor.matmul(
                                pv[:, hh * 512:(hh + 1) * 512],
                                xT8[:, 2 * g:2 * g + 2, ts],
                                wv_sb[:, 2 * g:2 * g + 2,
                                      hh * 512:(hh + 1) * 512],
                                start=(g == 0), stop=(g == Hc // 2 - 1),
                                perf_mode=PM.DoubleRow)
                    # Vn = pv/64 + 8*bv = 8*(V+bv), cast fp8, one DVE op
                    nc.vector.scalar_tensor_tensor(
                        out=Vn[:, t, :, 0:64],
                        in0=pv[:].rearrange("p (a b) -> p a b", a=NH),
                        scalar=1.0 / WSC,
                        in1=bv_bc.rearrange("p (a b) -> p a b", a=NH),
                        op0=ALU.mult, op1=ALU.add)

            with tc.tile_pool(name="wqkv", bufs=2) as wqkv, \
                 tc.tile_pool(name="rows", bufs=2) as rows, \
                 tc.tile_pool(name="psB", bufs=1, space="PSUM") as psB:

                def load_w(p):
                    sl = slice(p * Hc * 128, (p + 1) * Hc * 128)
                    wq_sb = wqkv.tile([128, Hc, 128], bf16, tag="wq",
                                      name="wq_sb")
                    dmac(out=wq_sb,
                         in_=WqR.ap()[:, sl].rearrange("k (c m) -> k c m",
                                                       c=Hc))
                    wk_sb = wqkv.tile([128, Hc, 128], fp8e4, tag="wk",
                                      name="wk_sb")
                    dmac(out=wk_sb,
                         in_=WkR.ap()[:, sl].rearrange("k (c m) -> k c m",
                                                       c=Hc))
                    return wq_sb, wk_sb

                w_next = load_w(0)
                dmac(out=bq_sb, in_=bq.ap().rearrange("(p k) -> k p", k=128))
                dmac(out=bk_sb, in_=bk.ap().rearrange("(p k) -> k p", k=128))
                dmac(out=b1_sb, in_=b1.ap().rearrange("(c k) -> k c", k=128))
                dmac(out=b2_sb, in_=b2.ap().rearrange("(c k) -> k c", k=128))
                dmac(out=l1g_sb,
                     in_=ln1_g.ap().rearrange("(c k) -> k c", k=128))
                dmac(out=l1b_sb,
                     in_=ln1_b.ap().rearrange("(c k) -> k c", k=128))
                dmac(out=l2g_sb,
                     in_=ln2_g.ap().rearrange("(c k) -> k c", k=128))
                dmac(out=l2b_sb,
                     in_=ln2_b.ap().rearrange("(c k) -> k c", k=128))

                def emit_scores_pair(QT, KT, qb):
                    qs = slice(qb * QB, (qb + 1) * QB)
                    e0 = wqkv.tile([128, Tkv // 2, 2, QB], edt, tag="expS0",
                                   bufs=3)
                    e1 = wqkv.tile([128, Tkv // 2, 2, QB], edt, tag="expS1",
                                   bufs=3)
                    for g in range(Tkv // 2):
                        ps0 = psB.tile([128, 2, QB], fp32, tag="ps", bufs=3)
                        ps1 = psB.tile([128, 2, QB], fp32, tag="ps", bufs=3)
                        for kk in range(2):
                            t = 2 * g + kk
                            ts = slice(t * 128, (t + 1) * 128)
                            # alternate heads: LDWEIGHTS of one row-group
                            # overlaps the other's matmul
                            nc.tensor.matmul(ps0[:, kk, :], KT[0:64, ts],
                                             QT[0:64, qs],
                                             start=True, stop=True)
                            nc.tensor.matmul(ps1[:, kk, :], KT[64:128, ts],
                                             QT[64:128, qs],
                                             start=True, stop=True)
                        nc.scalar.activation(out=e0[:, g, :, :], in_=ps0,
                                             func=AF.Exp, scale=0.125,
                                             bias=expb_sb)
                        nc.scalar.activation(out=e1[:, g, :, :], in_=ps1,
                                             func=AF.Exp, scale=0.125,
                                             bias=expb_sb)
                    return e0, e1

                def emit_ctx_pair(pp, h, eA, eB):
                    # both qb blocks against one LDWEIGHTS per (g, head)
                    head = 2 * pp + h
                    pcA = psB.tile([128, QB], fp32, tag="pcq", bufs=2)
                    pcB = psB.tile([128, QB], fp32, tag="pcq", bufs=2)
                    for g in range(Tkv // 2):
                        # lhsT [128, 2, 80]: dims 0:64 + ones@64 + zero pad
                        nc.tensor.matmul(
                            pcA[0:VW, :],
                            Vn[:, 2 * g:2 * g + 2, head, :],
                            eA[:, g, :, :],
                            start=(g == 0), stop=(g == Tkv // 2 - 1),
                            perf_mode=PM.DoubleRow)
                        nc.tensor.matmul(
                            pcB[0:VW, :],
                            Vn[:, 2 * g:2 * g + 2, head, :],
                            eB[:, g, :, :],
                            start=(g == 0), stop=(g == Tkv // 2 - 1),
                            perf_mode=PM.DoubleRow)
                    for qb, pc in ((0, pcA), (1, pcB)):
                        qs = slice(qb * QB, (qb + 1) * QB)
                        drow = rows.tile([1, QB], fp32, tag="drow")
                        nc.vector.tensor_copy(drow, pc[64:65, :])
                        frow = rows.tile([1, QB], fp32, tag="frow")
                        nc.vector.reciprocal_approx_fast(frow, drow)
                        rec = rows.tile([64, QB], fp32, tag="rec")
                        nc.gpsimd.partition_broadcast(rec, frow)
                        nc.vector.tensor_mul(
                            ctxT[h * 64:(h + 1) * 64, pp, qs],
                            pc[0:64, :], rec)

                def fold_residual(cp):
                    nc.vector.tensor_add(ctxT[:, cp, :], ctxT[:, cp, :],
                                         xT[:, cp, :])
                    nc.vector.tensor_mul(
                        sqall[:, cp, :, :],
                        ctxT[:, cp, :].rearrange("p (a b) -> p a b", a=QNB),
                        ctxT[:, cp, :].rearrange("p (a b) -> p a b", a=QNB))

                carry = None
                for p in range(NP):
                    QT = wqkv.tile([128, S_q], bf16, tag="QT", bufs=2)
                    KT = wqkv.tile([128, S_kv], bf16, tag="KT", bufs=2)
                    wq_sb, wk_sb = w_next
                    if p + 1 < NP:
                        w_next = load_w(p + 1)

                    for qb in range(QNB):
                        qs = slice(qb * NB, (qb + 1) * NB)
                        pq = psB.tile([128, NB], fp32, tag="pcq", bufs=2)
                        for c in range(Hc):
                            nc.tensor.matmul(
                                pq, wq_sb[:, c, :], xT[:, c, qs],
                                start=(c == 0), stop=(c == Hc - 1))
                        nc.vector.tensor_scalar(
                            out=QT[:, qs], in0=pq,
                            scalar1=bq_sb[:, p:p + 1], scalar2=None,
                            op0=ALU.add)
                    for kb in range(S_kv // NB):
                        qs = slice(kb * NB, (kb + 1) * NB)
                        pk = psB.tile([128, NB], fp32, tag="pcq", bufs=2)
                        for g in range(Hc // 2):
                            nc.tensor.matmul(
                                pk, wk_sb[:, 2 * g:2 * g + 2, :],
                                xT8[:, 2 * g:2 * g + 2, qs],
                                start=(g == 0), stop=(g == Hc // 2 - 1),
                                perf_mode=PM.DoubleRow)
                        nc.vector.tensor_scalar(
                            out=KT[:, qs], in0=pk,
                            scalar1=1.0 / (WSC * XSC),
                            scalar2=bk_sb[:, p:p + 1],
                            op0=ALU.mult, op1=ALU.add)

                    e0a, e1a = emit_scores_pair(QT, KT, 0)
                    if carry is not None:
                        cp, ce0a, ce1a, ce0b, ce1b = carry
                        emit_ctx_pair(cp, 0, ce0a, ce0b)
                        emit_ctx_pair(cp, 1, ce1a, ce1b)
                        fold_residual(cp)
                        carry = None
                    e0b, e1b = emit_scores_pair(QT, KT, 1)
                    carry = (p, e0a, e1a, e0b, e1b)

                # LN1 stats for chunks 0..6 (residuals already folded)
                # fill the PE gap while the last pair's exp backlog drains
                stA = psB.tile([128, 2, QB], fp32, tag="ps", bufs=3)
                stB = psB.tile([128, 2, QB], fp32, tag="ps", bufs=3)
                for c in range(Hc - 1):
                    for half, st in ((0, stA), (1, stB)):
                        qs = slice(half * NB, (half + 1) * NB)
                        nc.tensor.matmul(st[0:1, 0, :], ones_col,
                                         ctxT[:, c, qs],
                                         start=(c == 0), stop=False,
                                         skip_group_check=True)
                        nc.tensor.matmul(st[0:1, 1, :], ones_col_bf,
                                         sqall[:, c, half, :],
                                         start=(c == 0), stop=False,
                                         skip_group_check=True)
                cp, ce0a, ce1a, ce0b, ce1b = carry
                emit_ctx_pair(cp, 0, ce0a, ce0b)
                emit_ctx_pair(cp, 1, ce1a, ce1b)
                fold_residual(NP - 1)
                for half, st in ((0, stA), (1, stB)):
                    qs = slice(half * NB, (half + 1) * NB)
                    nc.tensor.matmul(st[0:1, 0, :], ones_col,
                                     ctxT[:, Hc - 1, qs],
                                     start=False, stop=True,
                                     skip_group_check=True)
                    nc.tensor.matmul(st[0:1, 1, :], ones_col_bf,
                                     sqall[:, Hc - 1, half, :],
                                     start=False, stop=True,
                                     skip_group_check=True)
                for half, st in ((0, stA), (1, stB)):
                    # reuse the ctx denominator row tags (same shape)
                    mu = rows.tile([1, QB], fp32, tag="drow")
                    msq = rows.tile([1, QB], fp32, tag="frow")
                    nc.vector.tensor_scalar_mul(mu, st[0:1, 0, :], 1.0 / H)
                    nc.gpsimd.partition_broadcast(pmu1[:, half, :], mu)
                    nc.vector.tensor_scalar_mul(msq, st[0:1, 1, :], 1.0 / H)
                    ve = rows.tile([1, QB], fp32, tag="drow")
                    nc.vector.tensor_mul(ve, mu, mu)
                    nc.vector.tensor_sub(ve, msq, ve)
                    nc.vector.tensor_scalar_add(ve, ve, EPS)
                    lnv = rows.tile([1, QB], fp32, tag="drow")
                    nc.scalar.activation(out=lnv, in_=ve, func=AF.Ln)
                    rstd = rows.tile([1, QB], fp32, tag="frow")
                    nc.scalar.activation(out=rstd, in_=lnv, func=AF.Exp,
                                         scale=-0.5)
                    nc.gpsimd.partition_broadcast(prs1[:, half, :], rstd)

        # -------- Phases C+D+E: LN1 + FFN + LN2 (all transposed) ---------
        with tc.tile_pool(name="lnpool", bufs=1) as lnpool, \
             tc.tile_pool(name="w1p", bufs=2) as w1p, \
             tc.tile_pool(name="w2p", bufs=2) as w2p, \
             tc.tile_pool(name="interp", bufs=1) as interp, \
             tc.tile_pool(name="stats", bufs=2) as stats, \
             tc.tile_pool(name="oster", bufs=3) as oster, \
             tc.tile_pool(name="psD", bufs=2, space="PSUM") as psD:
            ln1F = lnpool.tile([128, Hc, S_q], fdt, name="ln1F")
            interT = interp.tile([128, NFm, NB], fdt, name="interT")
            s2T = ctxT   # FFN accumulator aliases ctxT (per-half WAR)

            def emit_ln_stats(src_f32, src_sq, half, tag):
                """ones-matmul stats for a transposed layout tensor."""
                qs = slice(half * NB, (half + 1) * NB)
                psum = psD.tile([1, NB], fp32, tag="pst" + tag, bufs=1)
                psumsq = psD.tile([1, NB], fp32, tag="psq" + tag, bufs=1)
                for c in range(Hc):
                    nc.tensor.matmul(psum, ones_col, src_f32[:, c, qs],
                                     start=(c == 0), stop=(c == Hc - 1))
                for c in range(Hc):
                    nc.tensor.matmul(psumsq, ones_col_bf,
                                     src_sq[:, c, half, :],
                                     start=(c == 0), stop=(c == Hc - 1))
                mu = stats.tile([1, NB], fp32, tag="mu" + tag)
                rstd = stats.tile([1, NB], fp32, tag="rstd" + tag)
                msq = stats.tile([1, NB], fp32, tag="rowA" + tag)
                ve = stats.tile([1, NB], fp32, tag="rowB" + tag)
                nc.vector.tensor_scalar_mul(mu, psum, 1.0 / H)
                nc.vector.tensor_scalar_mul(msq, psumsq, 1.0 / H)
                nc.vector.tensor_mul(ve, mu, mu)
                nc.vector.tensor_sub(ve, msq, ve)
                nc.vector.tensor_scalar_add(ve, ve, EPS)
                # rstd = exp(-0.5*ln(v))
                lnv = stats.tile([1, NB], fp32, tag="rowD" + tag)
                nc.scalar.activation(out=lnv, in_=ve, func=AF.Ln)
                nc.scalar.activation(out=rstd, in_=lnv, func=AF.Exp,
                                     scale=-0.5)
                pmu = stats.tile([128, NB], fp32, tag="pmu" + tag)
                prs = stats.tile([128, NB], fp32, tag="prs" + tag)
                nc.gpsimd.partition_broadcast(pmu, mu)
                nc.gpsimd.partition_broadcast(prs, rstd)
                return pmu, prs

            def emit_ln1(half):
                qs = slice(half * NB, (half + 1) * NB)
                pmu = pmu1[:, half, :]
                prs = prs1[:, half, :]
                for c in range(Hc):
                    tmp_c = stats.tile([128, NB], fp32, tag="tmp")
                    nc.vector.tensor_sub(tmp_c, ctxT[:, c, qs], pmu)
                    nc.vector.tensor_mul(tmp_c, tmp_c, prs)
                    # gamma/beta affine on ACT (per-partition scale/bias)
                    nc.scalar.activation(
                        out=ln1F[:, c, qs], in_=tmp_c, func=AF.Identity,
                        bias=l1b_sb[:, c:c + 1], scale=l1g_sb[:, c:c + 1])
                    # seed FFN accumulator: s2 = ln1F + b2 (WAR on ctxT)
                    nc.scalar.activation(
                        out=s2T[:, c, qs], in_=ln1F[:, c, qs],
                        func=AF.Identity, bias=b2_sb[:, c:c + 1], scale=1.0)

            def emit_ffn_h1(half, interleave=None):
                qs = slice(half * NB, (half + 1) * NB)
                for fc in range(NFC):
                    w1_sb = w1p.tile([128, Hc, FFC], bf16, tag="w1")
                    dmac(out=w1_sb, in_=W1.ap()[:, fc * FFC:(fc + 1) * FFC]
                         .rearrange("(c k) f -> k c f", k=128))
                    for m in range(Fm):
                        fm = fc * Fm + m
                        pi = psD.tile([128, NB], fp32, tag="pi", bufs=2)
                        for c in range(Hc):
                            nc.tensor.matmul(
                                pi, w1_sb[:, c, m * 128:(m + 1) * 128],
                                ln1F[:, c, qs],
                                start=(c == 0), stop=(c == Hc - 1))
                        nc.scalar.activation(
                            out=interT[:, fm, :], in_=pi, func=act_fn,
                            bias=b1_sb[:, fm:fm + 1], scale=1.0)
                    if interleave is not None:
                        interleave(fc)

            def emit_ffn_h2(half, interleave=None):
                qs = slice(half * NB, (half + 1) * NB)
                for c in range(Hc):
                    sl = slice(c * NFm * 128, (c + 1) * NFm * 128)
                    w2_sb = w2p.tile([128, NFm, 128], fdt, tag="w2")
                    dmac(out=w2_sb,
                         in_=W2R.ap()[:, sl].rearrange("k (m n) -> k m n",
                                                       m=NFm))
                    ph = psD.tile([128, NB], fp32, tag="ph", bufs=2)
                    for m in range(NFm):
                        nc.tensor.matmul(ph, w2_sb[:, m, :], interT[:, m, :],
                                         start=(m == 0), stop=(m == NFm - 1))
                    nc.vector.tensor_add(s2T[:, c, qs], s2T[:, c, qs], ph)
                    # squares for the LN2 ones-matmul stats
                    nc.vector.tensor_mul(sqall[:, c, half, :],
                                         s2T[:, c, qs], s2T[:, c, qs])
                    if interleave is not None:
                        interleave(c)

            def emit_ln2(half):
                qs = slice(half * NB, (half + 1) * NB)
                pmu, prs = emit_ln_stats(s2T, sqall, half, "2")
                for c in range(Hc):
                    o_c = oster.tile([128, NB], fp32, tag="o_c", bufs=3)
                    nc.vector.tensor_sub(o_c, s2T[:, c, qs], pmu)
                    nc.vector.tensor_mul(o_c, o_c, prs)
                    nc.scalar.activation(
                        out=o_c, in_=o_c, func=AF.Identity,
                        bias=l2b_sb[:, c:c + 1], scale=l2g_sb[:, c:c + 1])
                    nc.sync.dma_start(
                        out=outT.ap()[c * 128:(c + 1) * 128, qs], in_=o_c)

            emit_ln1(0)

            def inter_d0(fc):
                if fc == 1:
                    emit_ln1(1)

            emit_ffn_h1(0, interleave=inter_d0)
            emit_ffn_h2(0)

            def inter_d1(fc):
                if fc == 1:
                    emit_ln2(0)

            emit_ffn_h1(1, interleave=inter_d1)
            emit_ffn_h2(1)
            emit_ln2(1)

    nc.compile()
    return nc


_CACHE = {}
TRACE = False
LAST_RESULT = None


def _get_nc(key, cfg):
    if key not in _CACHE:
        _CACHE[key] = build_nc(cfg)
    return _CACHE[key]


def kernel(hidden_states, Wq, bq, Wk, bk, Wv, bv, ln1_g, ln1_b,
           W1, b1, W2, b2, ln2_g, ln2_b):
    import ml_dtypes
    from concourse.bass_utils import run_bass_kernel_spmd

    B, S, H = hidden_states.shape
    cfg = FULL_CFG
    assert (B, S, H) == (4, 2048, 1024)
    nc = _get_nc("full", cfg)

    bfl = ml_dtypes.bfloat16
    f8 = ml_dtypes.float8_e4m3
    NP, Hc, NFm = 8, 8, 32

    def qk_pack(w):
        a = np.asarray(w, dtype=np.float32)
        a = a.reshape(Hc, 128, NP, 128).transpose(1, 2, 0, 3)
        return np.ascontiguousarray(a.reshape(128, -1).astype(bfl))

    def k_pack(w):
        a = np.clip(np.asarray(w, dtype=np.float32) * WSC, -240, 240)
        a = a.reshape(Hc, 128, NP, 128).transpose(1, 2, 0, 3)
        return np.ascontiguousarray(a.reshape(128, -1).astype(f8))

    shared = {"WqR": qk_pack(Wq), "WkR": k_pack(Wk)}
    shared["Wv8"] = np.ascontiguousarray(
        np.clip(np.asarray(Wv, dtype=np.float32) * WSC, -240, 240)
        .astype(f8))
    shared["W1"] = np.ascontiguousarray(
        np.asarray(W1, dtype=np.float32).astype(bfl))
    w2 = np.asarray(W2, dtype=np.float32)          # [FF, H]
    w2r = w2.reshape(NFm, 128, Hc, 128).transpose(1, 2, 0, 3)  # [k,c,m,n]
    shared["W2R"] = np.ascontiguousarray(w2r.reshape(128, -1).astype(bfl))
    for k, v in dict(bq=bq, bk=bk, bv=bv, b1=b1, b2=b2,
                     ln1_g=ln1_g, ln1_b=ln1_b,
                     ln2_g=ln2_g, ln2_b=ln2_b).items():
        shared[k] = np.ascontiguousarray(np.asarray(v, dtype=np.float32))

    hs = np.asarray(hidden_states, dtype=np.float32)

    in_maps = []
    for c in range(8):
        b, h = c // 2, c % 2
        xs = hs[b]
        rot = np.concatenate([xs[h * 1024:(h + 1) * 1024],
                              xs[(1 - h) * 1024:(2 - h) * 1024]], axis=0)
        rt = rot.T
        xbt = np.ascontiguousarray(rt[:, 0:1024].astype(bfl))
        x8t = np.ascontiguousarray(
            np.clip(rt * XSC, -240, 240).astype(f8))
        in_maps.append(dict(xbT=xbt, x8T=x8t, **shared))

    global LAST_RESULT
    try:
        res = run_bass_kernel_spmd(nc, in_maps, list(range(8)), trace=TRACE)
    except ModuleNotFoundError:
        res = run_bass_kernel_spmd(nc, in_maps, list(range(8)))
    LAST_RESULT = res
    outp = np.empty((4, 2048, 1024), dtype=np.float32)
    for c in range(8):
        b, h = c // 2, c % 2
        outp[b, h * 1024:(h + 1) * 1024] = res.results[c]["outT"].T
    return outp


# revision 34
# speedup vs baseline: 1.0169x; 1.0169x over previous
"""Trainium2 Bass kernel for a CamembertLayer (BERT encoder layer, no attn
output projection):  QKV -> attention -> +residual -> LN1 -> FFN(gelu) ->
+residual -> LN2.

Sharding: data-parallel over 8 cores.  Core c handles batch b=c//2, sequence
half h=c%2 (1024 query tokens).  K/V are computed redundantly over the full
2048-token sequence of the batch, so no collectives are needed.  The host
rotates each core's sequence so its query half is always rows 0..1023.

v3 design notes:
 - activations transposed ([H, tokens]); x^T arrives pre-transposed from the
   host in bf16 (xbT) and fp8 (x8T): no on-device x transposes at all.
 - Q/K projections in bf16 (fp8 scores cost ~1.5e-2 rel err: score errors
   scale with |s| and hit the softmax winners).  V is error-tolerant (damped
   by the residual) and is computed DIRECTLY in natural [kv, hd] layout for
   ALL heads upfront: fp8 DoubleRow with the x chunk as the stationary
   operand and Wv as the moving operand, accumulating [128kv x 8pairs*128]
   in two PSUM banks -- this kills the per-pair V^T projection, the PE
   V transposes, and their DVE copies, and frees 2 PSUM banks.
 - scores matmuls alternate heads tile-by-tile: the 64-row stationary
   (head-dim contraction) disables FWL, but alternating row-groups lets
   each LDWEIGHTS overlap the other head's matmul.  ps pool has 3 bufs so
   the ACT exp stream never starves.
 - exp with -3.6 bias (cancels in softmax; keeps fp8 < 240) -> fp8e4 probs;
   ctx matmul in fp8 DoubleRow against Vn (ones-column at col 64 folds the
   softmax denominator).
 - LN1/LN2 rstd = exp(-0.5*ln(var)) on ACT (exp/ln share one table set).
 - FFN: dense_1 streams W1 and gelu-writes the whole half's inter^T into
   one [128, 32, 512] buffer; dense_2 accumulates all 32 ff tiles into one
   PSUM bank per output chunk (one DVE add each).  W2 host-pre-rearranged.
 - LN2 runs in the transposed layout (ones-matmul stats like LN1) and the
   kernel emits a TRANSPOSED output [H, S_q]; the host transposes back.
   This removes the final PE transpose pass entirely.
"""
import sys

for _p in ("/opt/trn_rl_repo",):
    if _p not in sys.path:
        sys.path.insert(0, _p)

import numpy as np
from contextlib import ExitStack

import concourse.bass as bass
import concourse.bacc as bacc
import concourse.mybir as mybir
import concourse.tile as tile
from concourse.masks import make_identity

fp32 = mybir.dt.float32
fp32r = mybir.dt.float32r
bf16 = mybir.dt.bfloat16
fp8e4 = mybir.dt.float8e4
AF = mybir.ActivationFunctionType
ALU = mybir.AluOpType
PM = mybir.MatmulPerfMode

FULL_CFG = dict(H=1024, NH=16, FF=4096, S_kv=2048, S_q=1024, QB=512, FFC=512,
                act="gelu", ctx8=True)
EPS = 1e-12
HD = 64
EXPB = -3.6          # exp(s/8 + EXPB): cancels in softmax; keeps fp8 < 240
WSC = 64.0           # host scale on Wv fp8
XSC = 8.0            # host scale on x8T fp8


def build_nc(cfg):
    H, NH, FF = cfg["H"], cfg["NH"], cfg["FF"]
    S_kv, S_q, QB, FFC = cfg["S_kv"], cfg["S_q"], cfg["QB"], cfg["FFC"]
    Hc = H // 128          # hidden chunks of 128
    NP = NH // 2           # head pairs
    Tkv = S_kv // 128      # kv token tiles
    Tq = S_q // 128        # q token tiles
    NB = min(512, S_q)     # projection/stat block along q
    QNB = S_q // NB
    Fm = FFC // 128        # ff tiles per chunk
    NFC = FF // FFC        # ff chunks
    NFm = FF // 128        # all ff tiles
    act_fn = AF.Gelu if cfg.get("act", "gelu") == "gelu" else AF.Sigmoid
    ctx8 = cfg.get("ctx8", True)
    edt = fp8e4 if ctx8 else bf16       # expS / Vn dtype
    fdt = bf16

    nc = bacc.Bacc(num_swdge_queues=4)
    xbT = nc.declare_dram_parameter("xbT", [H, S_q], bf16, isOutput=False)
    x8T = nc.declare_dram_parameter("x8T", [H, S_kv], fp8e4, isOutput=False)
    WqR = nc.declare_dram_parameter("WqR", [128, NP * Hc * 128], bf16,
                                    isOutput=False)
    WkR = nc.declare_dram_parameter("WkR", [128, NP * Hc * 128], fp8e4,
                                    isOutput=False)
    Wv8 = nc.declare_dram_parameter("Wv8", [H, H], fp8e4, isOutput=False)
    bq = nc.declare_dram_parameter("bq", [H], fp32, isOutput=False)
    bk = nc.declare_dram_parameter("bk", [H], fp32, isOutput=False)
    bv = nc.declare_dram_parameter("bv", [H], fp32, isOutput=False)
    ln1_g = nc.declare_dram_parameter("ln1_g", [H], fp32, isOutput=False)
    ln1_b = nc.declare_dram_parameter("ln1_b", [H], fp32, isOutput=False)
    W1 = nc.declare_dram_parameter("W1", [H, FF], bf16, isOutput=False)
    b1 = nc.declare_dram_parameter("b1", [FF], fp32, isOutput=False)
    W2R = nc.declare_dram_parameter("W2R", [128, Hc * NFm * 128], fdt,
                                    isOutput=False)
    b2 = nc.declare_dram_parameter("b2", [H], fp32, isOutput=False)
    ln2_g = nc.declare_dram_parameter("ln2_g", [H], fp32, isOutput=False)
    ln2_b = nc.declare_dram_parameter("ln2_b", [H], fp32, isOutput=False)
    # transposed output [H, S_q]; host transposes back
    outT = nc.declare_dram_parameter("outT", [H, S_q], fp32, isOutput=True)

    dmac = nc.gpsimd.dma_start   # SWDGE: casts on the fly

    with tile.TileContext(nc) as tc, ExitStack() as ctx:
        pers = ctx.enter_context(tc.tile_pool(name="pers", bufs=1))

        ones_f = pers.tile([128, 128], fp32)
        nc.vector.memset(ones_f, 1.0)
        ones_col = pers.tile([128, 1], fp32r)
        nc.vector.tensor_copy(ones_col, ones_f[:, 0:1])
        ones_col_bf = pers.tile([128, 1], bf16)
        nc.vector.tensor_copy(ones_col_bf, ones_f[:, 0:1])
        expb_sb = pers.tile([128, 1], fp32, name="expb_sb")
        nc.vector.memset(expb_sb, EXPB)

        bq_sb = pers.tile([128, NP], fp32)
        bk_sb = pers.tile([128, NP], fp32)
        b1_sb = pers.tile([128, FF // 128], fp32)
        b2_sb = pers.tile([128, Hc], fp32)
        l1g_sb = pers.tile([128, Hc], fp32)
        l1b_sb = pers.tile([128, Hc], fp32)
        l2g_sb = pers.tile([128, Hc], fp32)
        l2b_sb = pers.tile([128, Hc], fp32)
        # persistent activations
        ctxT = pers.tile([128, Hc, S_q], fp32r)   # ctx^T, later s2^T
        sqall = pers.tile([128, Hc, QNB, NB], bf16)

        # ---------------- Phase A+B: x^T, V, attention -------------------
        with tc.tile_pool(name="attn", bufs=1) as attn:
            xT = attn.tile([128, Hc, S_q], bf16)
            xT8 = attn.tile([128, Hc, S_kv], fp8e4, name="xT8")
            VW = 80
            Vn = attn.tile([128, Tkv, NH, VW], edt, name="Vn")
            wv_sb = attn.tile([128, Hc, H], fp8e4, name="wv_sb")
            bv_bc = attn.tile([128, H], fp32, name="bv_bc")
            # DMA order: V-phase deps first (x8T + Wv), residual/Q x last,
            # per-chunk dmacs spread across the SWDGE queues
            for c in range(Hc):
                dmac(out=xT8[:, c, :],
                     in_=x8T.ap()[c * 128:(c + 1) * 128, :])
            for c in range(0, Hc, 2):
                dmac(out=wv_sb[:, c:c + 2, :],
                     in_=Wv8.ap()[c * 128:(c + 2) * 128, :]
                     .rearrange("(c k) m -> k c m", k=128))
            dmac(out=bv_bc, in_=bass.AP(tensor=bv, offset=0,
                                        ap=[[0, 128], [1, H]]))
            for c in range(Hc):
                dmac(out=xT[:, c, :],
                     in_=xbT.ap()[c * 128:(c + 1) * 128, :])
            # V natural for ALL heads, computed directly as [kv, head, dim]:
            # Vn[:, t, head, 0:64] = 8*(V+bv) dims, col 64 = 8.0 (denom
            # fold: ratio of the ctx matmul rows is invariant to the 8x),
            # cols 65:80 zero pad (stride 80 for the DoubleRow step rule)
            nc.vector.memset(Vn[:, :, :, 64:VW], 0.0)
            nc.vector.memset(Vn[:, :, :, 64:65], 8.0)
            nc.vector.tensor_scalar_mul(bv_bc, bv_bc, XSC)   # 8*bv

            with tc.tile_pool(name="psV", bufs=2, space="PSUM") as psV:
                for t in range(Tkv):
                    ts = slice(t * 128, (t + 1) * 128)
                    pv = psV.tile([128, 1024], fp32, tag="pv", bufs=2)
                    for g in range(Hc // 2):
                        # stationary = x chunk (fp8), moving = Wv columns
                        for hh in range(2):
                            nc.tensor.matmul(
                                pv[:, hh * 512:(hh + 1) * 512],
                                xT8[:, 2 * g:2 * g + 2, ts],
                                wv_sb[:, 2 * g:2 * g + 2,
                                      hh * 512:(hh + 1) * 512],
                                start=(g == 0), stop=(g == Hc // 2 - 1),
                                perf_mode=PM.DoubleRow)
                    # Vn = pv/64 + 8*bv = 8*(V+bv), cast fp8, one DVE op
                    nc.vector.scalar_tensor_tensor(
                        out=Vn[:, t, :, 0:64],
                        in0=pv[:].rearrange("p (a b) -> p a b", a=NH),
                        scalar=1.0 / WSC,
                        in1=bv_bc.rearrange("p (a b) -> p a b", a=NH),
                        op0=ALU.mult, op1=ALU.add)

            with tc.tile_pool(name="wqkv", bufs=2) as wqkv, \
                 tc.tile_pool(name="rows", bufs=2) as rows, \
                 tc.tile_pool(name="psB", bufs=1, space="PSUM") as psB:

                def load_w(p):
                    sl = slice(p * Hc * 128, (p + 1) * Hc * 128)
                    wq_sb = wqkv.tile([128, Hc, 128], bf16, tag="wq",
                                      name="wq_sb")
                    dmac(out=wq_sb,
                         in_=WqR.ap()[:, sl].rearrange("k (c m) -> k c m",
                                                       c=Hc))
                    wk_sb = wqkv.tile([128, Hc, 128], fp8e4, tag="wk",
                                      name="wk_sb")
                    dmac(out=wk_sb,
                         in_=WkR.ap()[:, sl].rearrange("k (c m) -> k c m",
                                                       c=Hc))
                    return wq_sb, wk_sb

                w_next = load_w(0)
                dmac(out=bq_sb, in_=bq.ap().rearrange("(p k) -> k p", k=128))
                dmac(out=bk_sb, in_=bk.ap().rearrange("(p k) -> k p", k=128))
                dmac(out=b1_sb, in_=b1.ap().rearrange("(c k) -> k c", k=128))
                dmac(out=b2_sb, in_=b2.ap().rearrange("(c k) -> k c", k=128))
                dmac(out=l1g_sb,
                     in_=ln1_g.ap().rearrange("(c k) -> k c", k=128))
                dmac(out=l1b_sb,
                     in_=ln1_b.ap().rearrange("(c k) -> k c", k=128))
                dmac(out=l2g_sb,
                     in_=ln2_g.ap().rearrange("(c k) -> k c", k=128))
                dmac(out=l2b_sb,
                     in_=ln2_b.ap().rearrange("(c k) -> k c", k=128))

                def emit_scores_pair(QT, KT, qb):
                    qs = slice(qb * QB, (qb + 1) * QB)
                    e0 = wqkv.tile([128, Tkv // 2, 2, QB], edt, tag="expS0",
                                   bufs=3)
                    e1 = wqkv.tile([128, Tkv // 2, 2, QB], edt, tag="expS1",
                                   bufs=3)
                    for g in range(Tkv // 2):
                        ps0 = psB.tile([128, 2, QB], fp32, tag="ps", bufs=3)
                        ps1 = psB.tile([128, 2, QB], fp32, tag="ps", bufs=3)
                        for kk in range(2):
                            t = 2 * g + kk
                            ts = slice(t * 128, (t + 1) * 128)
                            # alternate heads: LDWEIGHTS of one row-group
                            # overlaps the other's matmul
                            nc.tensor.matmul(ps0[:, kk, :], KT[0:64, ts],
                                             QT[0:64, qs],
                                             start=True, stop=True)
                            nc.tensor.matmul(ps1[:, kk, :], KT[64:128, ts],
                                             QT[64:128, qs],
                                             start=True, stop=True)
                        nc.scalar.activation(out=e0[:, g, :, :], in_=ps0,
                                             func=AF.Exp, scale=0.125,
                                             bias=expb_sb)
                        nc.scalar.activation(out=e1[:, g, :, :], in_=ps1,
                                             func=AF.Exp, scale=0.125,
                                             bias=expb_sb)
                    return e0, e1

                def emit_ctx_pair(pp, h, eA, eB):
                    # both qb blocks against one LDWEIGHTS per (g, head)
                    head = 2 * pp + h
                    pcA = psB.tile([128, QB], fp32, tag="pcq", bufs=2)
                    pcB = psB.tile([128, QB], fp32, tag="pcq", bufs=2)
                    for g in range(Tkv // 2):
                        # lhsT [128, 2, 80]: dims 0:64 + ones@64 + zero pad
                        nc.tensor.matmul(
                            pcA[0:VW, :],
                            Vn[:, 2 * g:2 * g + 2, head, :],
                            eA[:, g, :, :],
                            start=(g == 0), stop=(g == Tkv // 2 - 1),
                            perf_mode=PM.DoubleRow)
                        nc.tensor.matmul(
                            pcB[0:VW, :],
                            Vn[:, 2 * g:2 * g + 2, head, :],
                            eB[:, g, :, :],
                            start=(g == 0), stop=(g == Tkv // 2 - 1),
                            perf_mode=PM.DoubleRow)
                    for qb, pc in ((0, pcA), (1, pcB)):
                        qs = slice(qb * QB, (qb + 1) * QB)
                        drow = rows.tile([1, QB], fp32, tag="drow")
                        nc.vector.tensor_copy(drow, pc[64:65, :])
                        frow = rows.tile([1, QB], fp32, tag="frow")
                        nc.vector.reciprocal_approx_fast(frow, drow)
                        rec = rows.tile([64, QB], fp32, tag="rec")
                        nc.gpsimd.partition_broadcast(rec, frow)
                        nc.vector.tensor_mul(
                            ctxT[h * 64:(h + 1) * 64, pp, qs],
                            pc[0:64, :], rec)

                def fold_residual(cp):
                    nc.vector.tensor_add(ctxT[:, cp, :], ctxT[:, cp, :],
                                         xT[:, cp, :])
                    nc.vector.tensor_mul(
                        sqall[:, cp, :, :],
                        ctxT[:, cp, :].rearrange("p (a b) -> p a b", a=QNB),
                        ctxT[:, cp, :].rearrange("p (a b) -> p a b", a=QNB))

                carry = None
                for p in range(NP):
                    QT = wqkv.tile([128, S_q], bf16, tag="QT", bufs=2)
                    KT = wqkv.tile([128, S_kv], bf16, tag="KT", bufs=2)
                    wq_sb, wk_sb = w_next
                    if p + 1 < NP:
                        w_next = load_w(p + 1)

                    for qb in range(QNB):
                        qs = slice(qb * NB, (qb + 1) * NB)
                        pq = psB.tile([128, NB], fp32, tag="pcq", bufs=2)
                        for c in range(Hc):
                            nc.tensor.matmul(
                                pq, wq_sb[:, c, :], xT[:, c, qs],
                                start=(c == 0), stop=(c == Hc - 1))
                        nc.vector.tensor_scalar(
                            out=QT[:, qs], in0=pq,
                            scalar1=bq_sb[:, p:p + 1], scalar2=None,
                            op0=ALU.add)
                    for kb in range(S_kv // NB):
                        qs = slice(kb * NB, (kb + 1) * NB)
                        pk = psB.tile([128, NB], fp32, tag="pcq", bufs=2)
                        for g in range(Hc // 2):
                            nc.tensor.matmul(
                                pk, wk_sb[:, 2 * g:2 * g + 2, :],
                                xT8[:, 2 * g:2 * g + 2, qs],
                                start=(g == 0), stop=(g == Hc // 2 - 1),
                                perf_mode=PM.DoubleRow)
                        nc.vector.tensor_scalar(
                            out=KT[:, qs], in0=pk,
                            scalar1=1.0 / (WSC * XSC),
                            scalar2=bk_sb[:, p:p + 1],
                            op0=ALU.mult, op1=ALU.add)

                    e0a, e1a = emit_scores_pair(QT, KT, 0)
                    if carry is not None:
                        cp, ce0a, ce1a, ce0b, ce1b = carry
                        emit_ctx_pair(cp, 0, ce0a, ce0b)
                        emit_ctx_pair(cp, 1, ce1a, ce1b)
                        fold_residual(cp)
                        carry = None
                    e0b, e1b = emit_scores_pair(QT, KT, 1)
                    carry = (p, e0a, e1a, e0b, e1b)

                cp, ce0a, ce1a, ce0b, ce1b = carry
                emit_ctx_pair(cp, 0, ce0a, ce0b)
                emit_ctx_pair(cp, 1, ce1a, ce1b)
                fold_residual(NP - 1)

        # -------- Phases C+D+E: LN1 + FFN + LN2 (all transposed) ---------
        with tc.tile_pool(name="lnpool", bufs=1) as lnpool, \
             tc.tile_pool(name="w1p", bufs=2) as w1p, \
             tc.tile_pool(name="w2p", bufs=2) as w2p, \
             tc.tile_pool(name="interp", bufs=1) as interp, \
             tc.tile_pool(name="stats", bufs=2) as stats, \
             tc.tile_pool(name="oster", bufs=3) as oster, \
             tc.tile_pool(name="psD", bufs=2, space="PSUM") as psD:
            ln1F = lnpool.tile([128, Hc, S_q], fdt, name="ln1F")
            interT = interp.tile([128, NFm, NB], fdt, name="interT")
            s2T = ctxT   # FFN accumulator aliases ctxT (per-half WAR)

            def emit_ln_stats(src_f32, src_sq, half, tag):
                """ones-matmul stats for a transposed layout tensor."""
                qs = slice(half * NB, (half + 1) * NB)
                psum = psD.tile([1, NB], fp32, tag="pst" + tag, bufs=1)
                psumsq = psD.tile([1, NB], fp32, tag="psq" + tag, bufs=1)
                for c in range(Hc):
                    nc.tensor.matmul(psum, ones_col, src_f32[:, c, qs],
                                     start=(c == 0), stop=(c == Hc - 1))
                for c in range(Hc):
                    nc.tensor.matmul(psumsq, ones_col_bf,
                                     src_sq[:, c, half, :],
                                     start=(c == 0), stop=(c == Hc - 1))
                mu = stats.tile([1, NB], fp32, tag="mu" + tag)
                rstd = stats.tile([1, NB], fp32, tag="rstd" + tag)
                msq = stats.tile([1, NB], fp32, tag="rowA" + tag)
                ve = stats.tile([1, NB], fp32, tag="rowB" + tag)
                nc.vector.tensor_scalar_mul(mu, psum, 1.0 / H)
                nc.vector.tensor_scalar_mul(msq, psumsq, 1.0 / H)
                nc.vector.tensor_mul(ve, mu, mu)
                nc.vector.tensor_sub(ve, msq, ve)
                nc.vector.tensor_scalar_add(ve, ve, EPS)
                # rstd = exp(-0.5*ln(v))
                lnv = stats.tile([1, NB], fp32, tag="rowD" + tag)
                nc.scalar.activation(out=lnv, in_=ve, func=AF.Ln)
                nc.scalar.activation(out=rstd, in_=lnv, func=AF.Exp,
                                     scale=-0.5)
                pmu = stats.tile([128, NB], fp32, tag="pmu" + tag)
                prs = stats.tile([128, NB], fp32, tag="prs" + tag)
                nc.gpsimd.partition_broadcast(pmu, mu)
                nc.gpsimd.partition_broadcast(prs, rstd)
                return pmu, prs

            def emit_ln1(half):
                qs = slice(half * NB, (half + 1) * NB)
                pmu, prs = emit_ln_stats(ctxT, sqall, half, "1")
                for c in range(Hc):
                    tmp_c = stats.tile([128, NB], fp32, tag="tmp")
                    nc.vector.tensor_sub(tmp_c, ctxT[:, c, qs], pmu)
                    nc.vector.tensor_mul(tmp_c, tmp_c, prs)
                    # gamma/beta affine on ACT (per-partition scale/bias)
                    nc.scalar.activation(
                        out=ln1F[:, c, qs], in_=tmp_c, func=AF.Identity,
                        bias=l1b_sb[:, c:c + 1], scale=l1g_sb[:, c:c + 1])
                    # seed FFN accumulator: s2 = ln1F + b2 (WAR on ctxT)
                    nc.scalar.activation(
                        out=s2T[:, c, qs], in_=ln1F[:, c, qs],
                        func=AF.Identity, bias=b2_sb[:, c:c + 1], scale=1.0)

            def emit_ffn_h1(half, interleave=None):
                qs = slice(half * NB, (half + 1) * NB)
                for fc in range(NFC):
                    w1_sb = w1p.tile([128, Hc, FFC], bf16, tag="w1")
                    dmac(out=w1_sb, in_=W1.ap()[:, fc * FFC:(fc + 1) * FFC]
                         .rearrange("(c k) f -> k c f", k=128))
                    for m in range(Fm):
                        fm = fc * Fm + m
                        pi = psD.tile([128, NB], fp32, tag="pi", bufs=2)
                        for c in range(Hc):
                            nc.tensor.matmul(
                                pi, w1_sb[:, c, m * 128:(m + 1) * 128],
                                ln1F[:, c, qs],
                                start=(c == 0), stop=(c == Hc - 1))
                        nc.scalar.activation(
                            out=interT[:, fm, :], in_=pi, func=act_fn,
                            bias=b1_sb[:, fm:fm + 1], scale=1.0)
                    if interleave is not None:
                        interleave(fc)

            def emit_ffn_h2(half, interleave=None):
                qs = slice(half * NB, (half + 1) * NB)
                for c in range(Hc):
                    sl = slice(c * NFm * 128, (c + 1) * NFm * 128)
                    w2_sb = w2p.tile([128, NFm, 128], fdt, tag="w2")
                    dmac(out=w2_sb,
                         in_=W2R.ap()[:, sl].rearrange("k (m n) -> k m n",
                                                       m=NFm))
                    ph = psD.tile([128, NB], fp32, tag="ph", bufs=2)
                    for m in range(NFm):
                        nc.tensor.matmul(ph, w2_sb[:, m, :], interT[:, m, :],
                                         start=(m == 0), stop=(m == NFm - 1))
                    nc.vector.tensor_add(s2T[:, c, qs], s2T[:, c, qs], ph)
                    # squares for the LN2 ones-matmul stats
                    nc.vector.tensor_mul(sqall[:, c, half, :],
                                         s2T[:, c, qs], s2T[:, c, qs])
                    if interleave is not None:
                        interleave(c)

            def emit_ln2(half):
                qs = slice(half * NB, (half + 1) * NB)
                pmu, prs = emit_ln_stats(s2T, sqall, half, "2")
                for c in range(Hc):
                    o_c = oster.tile([128, NB], fp32, tag="o_c", bufs=3)
                    nc.vector.tensor_sub(o_c, s2T[:, c, qs], pmu)
                    nc.vector.tensor_mul(o_c, o_c, prs)
                    nc.scalar.activation(
                        out=o_c, in_=o_c, func=AF.Identity,
                        bias=l2b_sb[:, c:c + 1], scale=l2g_sb[:, c:c + 1])
                    nc.sync.dma_start(
                        out=outT.ap()[c * 128:(c + 1) * 128, qs], in_=o_c)

            emit_ln1(0)

            def inter_d0(fc):
                if fc == 1:
                    emit_ln1(1)

            emit_ffn_h1(0, interleave=inter_d0)
            emit_ffn_h2(0)

            def inter_d1(fc):
                if fc == 1:
                    emit_ln2(0)

            emit_ffn_h1(1, interleave=inter_d1)
            emit_ffn_h2(1)
            emit_ln2(1)

    nc.compile()
    return nc


_CACHE = {}
TRACE = False
LAST_RESULT = None


def _get_nc(key, cfg):
    if key not in _CACHE:
        _CACHE[key] = build_nc(cfg)
    return _CACHE[key]


def kernel(hidden_states, Wq, bq, Wk, bk, Wv, bv, ln1_g, ln1_b,
           W1, b1, W2, b2, ln2_g, ln2_b):
    import ml_dtypes
    from concourse.bass_utils import run_bass_kernel_spmd

    B, S, H = hidden_states.shape
    cfg = FULL_CFG
    assert (B, S, H) == (4, 2048, 1024)
    nc = _get_nc("full", cfg)

    bfl = ml_dtypes.bfloat16
    f8 = ml_dtypes.float8_e4m3
    NP, Hc, NFm = 8, 8, 32

    def qk_pack(w):
        a = np.asarray(w, dtype=np.float32)
        a = a.reshape(Hc, 128, NP, 128).transpose(1, 2, 0, 3)
        return np.ascontiguousarray(a.reshape(128, -1).astype(bfl))

    def k_pack(w):
        a = np.clip(np.asarray(w, dtype=np.float32) * WSC, -240, 240)
        a = a.reshape(Hc, 128, NP, 128).transpose(1, 2, 0, 3)
        return np.ascontiguousarray(a.reshape(128, -1).astype(f8))

    shared = {"WqR": qk_pack(Wq), "WkR": k_pack(Wk)}
    shared["Wv8"] = np.ascontiguousarray(
        np.clip(np.asarray(Wv, dtype=np.float32) * WSC, -240, 240)
        .astype(f8))
    shared["W1"] = np.ascontiguousarray(
        np.asarray(W1, dtype=np.float32).astype(bfl))
    w2 = np.asarray(W2, dtype=np.float32)          # [FF, H]
    w2r = w2.reshape(NFm, 128, Hc, 128).transpose(1, 2, 0, 3)  # [k,c,m,n]
    shared["W2R"] = np.ascontiguousarray(w2r.reshape(128, -1).astype(bfl))
    for k, v in dict(bq=bq, bk=bk, bv=bv, b1=b1, b2=b2,
                     ln1_g=ln1_g, ln1_b=ln1_b,
                     ln2_g=ln2_g, ln2_b=ln2_b).items():
        shared[k] = np.ascontiguousarray(np.asarray(v, dtype=np.float32))

    hs = np.asarray(hidden_states, dtype=np.float32)

    in_maps = []
    for c in range(8):
        b, h = c // 2, c % 2
        xs = hs[b]
        rot = np.concatenate([xs[h * 1024:(h + 1) * 1024],
                              xs[(1 - h) * 1024:(2 - h) * 1024]], axis=0)
        rt = rot.T
        xbt = np.ascontiguousarray(rt[:, 0:1024].astype(bfl))
        x8t = np.ascontiguousarray(
            np.clip(rt * XSC, -240, 240).astype(f8))
        in_maps.append(dict(xbT=xbt, x8T=x8t, **shared))

    global LAST_RESULT
    try:
        res = run_bass_kernel_spmd(nc, in_maps, list(range(8)), trace=TRACE)
    except ModuleNotFoundError:
        res = run_bass_kernel_spmd(nc, in_maps, list(range(8)))
    LAST_RESULT = res
    outp = np.empty((4, 2048, 1024), dtype=np.float32)
    for c in range(8):
        b, h = c // 2, c % 2
        outp[b, h * 1024:(h + 1) * 1024] = res.results[c]["outT"].T
    return outp
